# revision 27
# baseline (speedup 1.0000x reference)
"""Grid-MERA (16x16 in, bond 4) Trainium2 kernel — bf16 edition.

Strategy: model-parallel shard of the dominant tensor e1 [16,16,16,16,256]
along its first leg 'a' -> 2 values / core across 8 cores. Every core
computes a partial v (the e1 contraction) for the FULL batch, then runs the
(v-linear) remainder of the network; the full output is the sum of the 8
per-core partial outputs — no collectives.

All weights are cast to bf16 on the host (tolerance 2e-2 >> bf16 noise),
halving the dominant HBM stream and running the PE at bf16 rate. e1 is
pre-transposed on the host into the exact SBUF layout so each DMA partition
line is contiguous in DRAM.

Per-core pipeline (all on device):
  1. Gcd = p9 (x) p10 rank-1 product               (DVE)
  2. P6D block-diagonal built on device from IDN   (DVE)
  3. H^T[(b,cd), B] via PE "diag trick": lhsT=Gcd half, rhs=P6D chunk
     -> transposed AND p6-scaled in one matmul; ACT copies cast to bf16
  4. isometries: G_i = pj (x) pk; PE transpose; Y = G^T.T @ ISO;
     z = Y * pout; tall = reduce_i z   (t-matrices, stored (o, m))
  5. v[B, (a|fghe)] = sum_{b,h} HT_ch.T @ E1_ch    (PE, one PSUM bank)
  6. tail: 4 per-sample mode products, each ONE tensor_tensor (2x bf16
     mode, pre-expanded t-operands) + ONE tensor_reduce
  7. out_partial = w^T.T @ M + bias   (M = e2@iso2 precomputed on host)
"""

import numpy as np
import ml_dtypes

import concourse.bass as bass
import concourse.bacc as bacc
import concourse.tile as tile
import concourse.mybir as mybir
from concourse import bass_utils

F32 = mybir.dt.float32
BF16 = mybir.dt.bfloat16
NPBF16 = ml_dtypes.bfloat16
B = 128
NCORES = 8

# blobA column map (everything small; lands first)
XB_C = 0           # 256
IDN_C = 256        # 128
M_C = 384          # 2 x 10
P5_C = 404         # 2
BIAS_C = 406       # 10
BA_W = 416
# blobISO column map (needed only by the Y matmuls; lands mid-stream)
BB_W = 2048

_CACHE = {}


def _build_nc():
    nc = bacc.Bacc(
        "TRN2", target_bir_lowering=False, debug=False, num_devices=NCORES
    )

    dBA = nc.dram_tensor("BA", [B, BA_W], BF16, kind="ExternalInput")
    dBB = nc.dram_tensor("BB", [B, BB_W], BF16, kind="ExternalInput")
    dE1 = nc.dram_tensor("E1", [B, 16384], BF16, kind="ExternalInput")
    dOUT = nc.dram_tensor("OUT", [B, 10], F32, kind="ExternalOutput")

    mul = mybir.AluOpType.mult
    add = mybir.AluOpType.add

    with tile.TileContext(nc) as tc, nc.allow_low_precision(
        reason="bf16 pipeline; tolerance 2e-2"
    ):
        with (
            tc.tile_pool(name="sb", bufs=1) as sb,
            tc.tile_pool(name="psH", bufs=2, space="PSUM") as psH,
            tc.tile_pool(name="psT", bufs=2, space="PSUM") as psT,
            tc.tile_pool(name="psY", bufs=1, space="PSUM") as psY,
            tc.tile_pool(name="psV", bufs=1, space="PSUM") as psV,
        ):
            ba = sb.tile([B, BA_W], BF16, tag="ba")
            bb = sb.tile([B, BB_W], BF16, tag="bb")
            e1g = [
                sb.tile([B, 4096], BF16, tag=f"e1g{g}", name=f"e1g{g}")
                for g in range(4)
            ]
            p6d = sb.tile([B, 2048], BF16, tag="p6d")
            gcd = sb.tile([B, 256], BF16, tag="gcd")
            gi = sb.tile([B, 1024], BF16, tag="gi")
            ht = sb.tile([B, 4096], BF16, tag="ht")
            gt = sb.tile([B, 1024], BF16, tag="gt")
            zy = sb.tile([B, 1024], BF16, tag="zy")
            tall = sb.tile([B, 64], BF16, tag="tall")
            tx = [
                sb.tile([B, 1024], BF16, tag=f"tx{s}", name=f"tx{s}")
                for s in range(4)
            ]
            p5f = sb.tile([B, 2], F32, tag="p5f")
            vtmp = sb.tile([B, 256], BF16, tag="vtmp")
            vsb = sb.tile([B, 256], BF16, tag="vsb")
            zt = sb.tile([B, 1024], BF16, tag="zt")
            a1 = sb.tile([B, 256], BF16, tag="a1")
            a2 = sb.tile([B, 256], BF16, tag="a2")
            a3 = sb.tile([B, 256], BF16, tag="a3")
            wsb = sb.tile([B, 256], BF16, tag="wsb")
            wt = sb.tile([B, 256], BF16, tag="wt")
            outsb = sb.tile([B, 10], F32, tag="outsb")

            idn = ba[:, IDN_C:IDN_C + 128]

            def patch(k):
                return ba[:, XB_C + k * 16:XB_C + (k + 1) * 16]

            # ---------- input DMAs ----------
            nc.sync.dma_start(ba[:], dBA[:, :])
            nc.sync.dma_start(e1g[0][:], dE1[:, 0:4096])
            nc.sync.dma_start(bb[:], dBB[:, :])
            for g in range(1, 3):
                nc.sync.dma_start(
                    e1g[g][:], dE1[:, g * 4096:(g + 1) * 4096]
                )
            # last group split fine so the tail-end accumulating matmuls can
            # chase the wire and fire almost immediately at wire-end
            for p in range(4):
                nc.sync.dma_start(
                    e1g[3][:, p * 1024:(p + 1) * 1024],
                    dE1[:, 12288 + p * 1024:12288 + (p + 1) * 1024],
                )

            # ---------- PE warm-up: release the HAM clock gate early -------
            wu = sb.tile([B, 128], BF16, tag="wu")
            nc.gpsimd.memset(wu[:], 0)
            wup = psH.tile([B, 512], F32, tag="hta", name="wup")
            for _ in range(30):
                nc.tensor.matmul(
                    wup[:, 0:128], wu[:], wu[:], start=True, stop=True
                )

            # ---------- Gcd = p9 (x) p10 ----------
            nc.vector.tensor_tensor(
                gcd[:].rearrange("p (c d) -> p c d", c=16),
                patch(9).unsqueeze(2).broadcast_to((B, 16, 16)),
                patch(10).unsqueeze(1).broadcast_to((B, 16, 16)),
                mul,
            )

            # P6D[s, b*128+j] = p6[s,b] * IDN[s,j], built in quarters
            def p6d_quarter(q):
                nc.vector.tensor_tensor(
                    p6d[:, q * 512:(q + 1) * 512].rearrange(
                        "p (b j) -> p b j", b=4
                    ),
                    idn.unsqueeze(1).broadcast_to((B, 4, 128)),
                    ba[:, XB_C + 96 + 4 * q:XB_C + 100 + 4 * q]
                    .unsqueeze(2)
                    .broadcast_to((B, 4, 128)),
                    mul,
                )

            # H^T via diag trick (PE); cast-copies split ACT / GpSimd
            def diag_quarter(q):
                for h in range(2):
                    pst = psH.tile([B, 512], F32, tag="hta")
                    nc.tensor.matmul(
                        pst[:],
                        gcd[:, h * 128:(h + 1) * 128],
                        p6d[:, q * 512:(q + 1) * 512],
                        start=True,
                        stop=True,
                    )
                    # scatter 4 b-blocks to chunk slots ch=2*(4q+r)+h
                    dst = bass.AP(
                        ht.tensor,
                        ht[:].offset + ((8 * q + h) * 128),
                        [ht[:].ap[0], [256, 4], [1, 128]],
                    )
                    src = pst[:].rearrange("p (r j) -> p r j", r=4)
                    nc.scalar.copy(dst, src)

            # ---------- iso stage: G_i builds ----------
            G_pairs = [(1, 4), (3, 7), (12, 13), (14, 15)]
            out_patches = [0, 2, 8, 11]
            for s, (pj, pk) in enumerate(G_pairs):
                nc.vector.tensor_tensor(
                    gi[:, s * 256:(s + 1) * 256].rearrange(
                        "p (j k) -> p j k", j=16
                    ),
                    patch(pj).unsqueeze(2).broadcast_to((B, 16, 16)),
                    patch(pk).unsqueeze(1).broadcast_to((B, 16, 16)),
                    mul,
                )

            p6d_quarter(0)
            diag_quarter(0)

            # ---------- G_i^T via PE transpose (batched 4 per PSUM tile) ---
            for t in range(2):
                pst = psT.tile([B, 512], BF16, tag="pst")
                for k in range(4):
                    nc.tensor.transpose(
                        pst[:, k * 128:(k + 1) * 128],
                        gi[:, (t * 4 + k) * 128:(t * 4 + k + 1) * 128],
                        idn,
                    )
                nc.scalar.copy(gt[:, t * 512:(t + 1) * 512], pst[:])

            p6d_quarter(1)
            diag_quarter(1)
            p6d_quarter(2)
            diag_quarter(2)
            p6d_quarter(3)
            diag_quarter(3)

            # ---------- Y matmuls ----------
            py = psY.tile([B, 1024], F32, tag="py")
            for s in range(4):
                for ch in range(2):
                    nc.tensor.matmul(
                        py[:, s * 256:(s + 1) * 256],
                        gt[:, (s * 2 + ch) * 128:(s * 2 + ch + 1) * 128],
                        bb[:, ch * 1024 + s * 256:ch * 1024 + (s + 1) * 256],
                        start=(ch == 0),
                        stop=(ch == 1),
                    )
            # ---------- z = Y * pout ; tall = reduce_i z  (layout (o,m)) ---
            for s, po in enumerate(out_patches):
                nc.vector.tensor_tensor(
                    zy[:, s * 256:(s + 1) * 256].rearrange(
                        "p (o m i) -> p o m i", o=4, m=4
                    ),
                    py[:, s * 256:(s + 1) * 256].rearrange(
                        "p (i m o) -> p o m i", i=16, m=4
                    ),
                    patch(po)
                    .unsqueeze(1)
                    .unsqueeze(1)
                    .broadcast_to((B, 4, 4, 16)),
                    mul,
                )
            nc.vector.tensor_reduce(
                tall[:],
                zy[:].rearrange("p (x i) -> p x i", i=16),
                axis=mybir.AxisListType.X,
                op=add,
            )
            # ---------- expansions: tx[s][p, (o, 64, m)] = t_s[m, o] -------
            nc.vector.tensor_copy(p5f[:], ba[:, P5_C:P5_C + 2])
            for s in range(4):
                src = bass.AP(
                    tall.tensor,
                    tall[:].offset + s * 16,
                    [tall[:].ap[0], [4, 4], [0, 64], [1, 4]],
                )
                dst = tx[s][:].rearrange("p (o x m) -> p o x m", o=4, x=64)
                nc.vector.tensor_copy(dst, src)

            # ---------- big matmul: pv[B, (a|fghe)] ------------------------
            pv = psV.tile([B, 512], F32, tag="pv")
            for g in range(4):
                for j in range(8):
                    nc.tensor.matmul(
                        pv[:],
                        ht[:, (8 * g + j) * 128:(8 * g + j + 1) * 128],
                        e1g[g][:, j * 512:(j + 1) * 512],
                        start=(g == 0 and j == 0),
                        stop=(g == 3 and j == 7),
                    )
            # v = p5[:,0]*v0 + p5[:,1]*v1
            nc.vector.tensor_scalar(
                vtmp[:], pv[:, 0:256], p5f[:, 0:1], None, mul
            )
            nc.vector.scalar_tensor_tensor(
                vsb[:], pv[:, 256:512], p5f[:, 1:2], vtmp[:],
                mul, add,
            )

            # ---------- tail: 4 mode products, all 2x-eligible -------------
            # each: Z[p, new, k, old] = A[p, k, old] * t[old, new]; reduce old
            def mode_step(dst, src, s):
                nc.vector.tensor_tensor(
                    zt[:].rearrange("p (n x o) -> p n x o", n=4, x=64),
                    src[:].rearrange("p (x o) -> p x o", o=4)
                    .unsqueeze(1)
                    .broadcast_to((B, 4, 64, 4)),
                    tx[s][:].rearrange("p (n x o) -> p n x o", n=4, x=64),
                    mul,
                )
                nc.vector.tensor_reduce(
                    dst[:],
                    zt[:].rearrange("p (x o) -> p x o", o=4),
                    axis=mybir.AxisListType.X,
                    op=add,
                )

            mode_step(a1, vsb, 0)   # contract e -> a ; A1 (a,f,g,h)
            mode_step(a2, a1, 3)    # contract h -> d ; A2 (d,a,f,g)
            mode_step(a3, a2, 2)    # contract g -> c ; A3 (c,d,a,f)
            mode_step(wsb, a3, 1)   # contract f -> b ; w  (b,c,d,a)

            # ---------- final: out = w^T.T @ M + bias ----------------------
            pwt = psT.tile([B, 512], BF16, tag="pst", name="pwt")
            for hf in range(2):
                nc.tensor.transpose(
                    pwt[:, hf * 128:(hf + 1) * 128],
                    wsb[:, hf * 128:(hf + 1) * 128],
                    idn,
                )
            nc.scalar.copy(wt[:], pwt[:, 0:256])
            po_ = psT.tile([B, 512], F32, tag="pst", name="po")
            for hf in range(2):
                nc.tensor.matmul(
                    po_[:, 0:10],
                    wt[:, hf * 128:(hf + 1) * 128],
                    ba[:, M_C + hf * 10:M_C + (hf + 1) * 10],
                    start=(hf == 0),
                    stop=(hf == 1),
                )
            nc.vector.tensor_tensor(
                outsb[:], po_[:, 0:10], ba[:, BIAS_C:BIAS_C + 10], add
            )
            nc.sync.dma_start(dOUT[:, :], outsb[:])

    nc.compile()
    return nc


def _host_prep(inputs, e1, e2, iso1_0, iso1_1, iso1_2, iso1_3, iso2, bias):
    """Layout prep: patch extraction, permutes, bf16 casts, per-core slices."""
    x = np.ascontiguousarray(np.asarray(inputs, np.float32))
    xv = (
        x[..., 0]
        .reshape(B, 4, 4, 4, 4)
        .transpose(0, 1, 3, 2, 4)
        .reshape(B, 16, 16)
    )
    Xb = xv.reshape(B, 256).astype(NPBF16)
    IDNb = np.eye(B, dtype=NPBF16)

    iso_perms = [
        (np.asarray(iso1_0, np.float32), (1, 2, 0, 3, 4)),
        (np.asarray(iso1_1, np.float32), (1, 3, 0, 2, 4)),
        (np.asarray(iso1_2, np.float32), (2, 3, 0, 1, 4)),
        (np.asarray(iso1_3, np.float32), (2, 3, 1, 0, 4)),
    ]
    ISO = np.concatenate(
        [t.transpose(p).reshape(256, 256) for t, p in iso_perms], axis=1
    ).astype(NPBF16)  # [256 (jk), 1024 (s, i, m, o)]

    M = (
        np.asarray(e2, np.float32).reshape(256, 256)
        @ np.asarray(iso2, np.float32).reshape(256, 10)
    )  # rows (a,b,c,d) -> permute to w layout (b,c,d,a)
    Mp = (
        M.reshape(4, 4, 4, 4, 10).transpose(1, 2, 3, 0, 4).reshape(256, 10)
    ).astype(NPBF16)

    # E1: [a,b,c,d,e,f,g,h] -> per-core [p=cd%128, cols g|j|a|f,g,h,e]
    E = np.asarray(e1, np.float32).reshape(8, 2, 4, 4, 2, 128, 4, 4, 4, 4)
    # axes: core, ar, bhi, blo, hh, p, e, f, g, h
    E = E.transpose(0, 5, 2, 3, 4, 1, 7, 8, 9, 6)
    E1all = np.ascontiguousarray(E).reshape(8, 128, 16384).astype(NPBF16)

    biasv = np.asarray(bias, np.float32).reshape(10)
    p5 = xv[:, 5, :]  # [B, 16]

    BB = np.ascontiguousarray(
        np.concatenate([ISO[0:128], ISO[128:256]], axis=1)
    )  # [128, 2048]

    in_maps = []
    for core in range(NCORES):
        a0 = 2 * core
        bac = np.zeros((B, BA_W), NPBF16)
        bac[:, XB_C:XB_C + 256] = Xb
        bac[:, IDN_C:IDN_C + 128] = IDNb
        bac[:, M_C:M_C + 10] = Mp[0:128]
        bac[:, M_C + 10:M_C + 20] = Mp[128:256]
        bac[:, P5_C:P5_C + 2] = p5[:, a0:a0 + 2].astype(NPBF16)
        if core == 0:
            bac[:, BIAS_C:BIAS_C + 10] = np.tile(biasv, (B, 1)).astype(
                NPBF16
            )
        in_maps.append(
            {
                "BA": np.ascontiguousarray(bac),
                "BB": BB,
                "E1": np.ascontiguousarray(E1all[core]),
            }
        )
    return in_maps


def kernel(inputs, e1, e2, iso1_0, iso1_1, iso1_2, iso1_3, iso2, bias):
    if "nc" not in _CACHE:
        _CACHE["nc"] = _build_nc()
    nc = _CACHE["nc"]
    in_maps = _host_prep(
        inputs, e1, e2, iso1_0, iso1_1, iso1_2, iso1_3, iso2, bias
    )
    res = bass_utils.run_bass_kernel_spmd(
        nc, in_maps, core_ids=list(range(NCORES))
    )
    out = np.zeros((B, 10), np.float32)
    for core in range(NCORES):
        out = out + res.results[core]["OUT"]
    return out.astype(np.float32)


# revision 28
# speedup vs baseline: 1.0663x; 1.0663x over previous
"""Grid-MERA (16x16 in, bond 4) Trainium2 kernel — bf16 edition.

Strategy: model-parallel shard of the dominant tensor e1 [16,16,16,16,256]
along its first leg 'a' -> 2 values / core across 8 cores. Every core
computes a partial v (the e1 contraction) for the FULL batch, then runs the
(v-linear) remainder of the network; the full output is the sum of the 8
per-core partial outputs — no collectives.

All weights are cast to bf16 on the host (tolerance 2e-2 >> bf16 noise),
halving the dominant HBM stream and running the PE at bf16 rate. e1 is
pre-transposed on the host into the exact SBUF layout so each DMA partition
line is contiguous in DRAM.

Per-core pipeline (all on device):
  1. Gcd = p9 (x) p10 rank-1 product               (DVE)
  2. P6D block-diagonal built on device from IDN   (DVE)
  3. H^T[(b,cd), B] via PE "diag trick": lhsT=Gcd half, rhs=P6D chunk
     -> transposed AND p6-scaled in one matmul; ACT copies cast to bf16
  4. isometries: G_i = pj (x) pk; PE transpose; Y = G^T.T @ ISO;
     z = Y * pout; tall = reduce_i z   (t-matrices, stored (o, m))
  5. v[B, (a|fghe)] = sum_{b,h} HT_ch.T @ E1_ch    (PE, one PSUM bank)
  6. tail: 4 per-sample mode products, each ONE tensor_tensor (2x bf16
     mode, pre-expanded t-operands) + ONE tensor_reduce
  7. out_partial = w^T.T @ M + bias   (M = e2@iso2 precomputed on host)
"""

import numpy as np
import ml_dtypes

import concourse.bass as bass
import concourse.bacc as bacc
import concourse.tile as tile
import concourse.mybir as mybir
from concourse import bass_utils

F32 = mybir.dt.float32
BF16 = mybir.dt.bfloat16
NPBF16 = ml_dtypes.bfloat16
B = 128
NCORES = 8

# blobA column map (everything small; lands first)
XB_C = 0           # 256
IDN_C = 256        # 128
M_C = 384          # 2 x 10
P5_C = 404         # 2
BIAS_C = 406       # 10
BA_W = 416
# blobISO column map (needed only by the Y matmuls; lands mid-stream)
BB_W = 2048

_CACHE = {}


def _build_nc():
    nc = bacc.Bacc(
        "TRN2", target_bir_lowering=False, debug=False, num_devices=NCORES
    )

    dBA = nc.dram_tensor("BA", [B, BA_W], BF16, kind="ExternalInput")
    dBB = nc.dram_tensor("BB", [B, BB_W], BF16, kind="ExternalInput")
    dE1 = nc.dram_tensor("E1", [B, 16384], BF16, kind="ExternalInput")
    dOUT = nc.dram_tensor("OUT", [B, 10], F32, kind="ExternalOutput")

    mul = mybir.AluOpType.mult
    add = mybir.AluOpType.add

    with tile.TileContext(nc) as tc, nc.allow_low_precision(
        reason="bf16 pipeline; tolerance 2e-2"
    ):
        with (
            tc.tile_pool(name="sb", bufs=1) as sb,
            tc.tile_pool(name="psH", bufs=2, space="PSUM") as psH,
            tc.tile_pool(name="psT", bufs=2, space="PSUM") as psT,
            tc.tile_pool(name="psY", bufs=1, space="PSUM") as psY,
            tc.tile_pool(name="psV", bufs=1, space="PSUM") as psV,
        ):
            ba = sb.tile([B, BA_W], BF16, tag="ba")
            bb = sb.tile([B, BB_W], BF16, tag="bb")
            e1g = [
                sb.tile([B, 4096], BF16, tag=f"e1g{g}", name=f"e1g{g}")
                for g in range(4)
            ]
            p6d = sb.tile([B, 2048], BF16, tag="p6d")
            gcd = sb.tile([B, 256], BF16, tag="gcd")
            gi = sb.tile([B, 1024], BF16, tag="gi")
            ht = sb.tile([B, 4096], BF16, tag="ht")
            gt = sb.tile([B, 1024], BF16, tag="gt")
            zy = sb.tile([B, 1024], BF16, tag="zy")
            tall = sb.tile([B, 64], BF16, tag="tall")
            tx = [
                sb.tile([B, 1024], BF16, tag=f"tx{s}", name=f"tx{s}")
                for s in range(4)
            ]
            p5f = sb.tile([B, 2], F32, tag="p5f")
            vtmp = sb.tile([B, 256], BF16, tag="vtmp")
            vsb = sb.tile([B, 256], BF16, tag="vsb")
            zt = sb.tile([B, 1024], BF16, tag="zt")
            a1 = sb.tile([B, 256], BF16, tag="a1")
            a2 = sb.tile([B, 256], BF16, tag="a2")
            a3 = sb.tile([B, 256], BF16, tag="a3")
            wsb = sb.tile([B, 256], BF16, tag="wsb")
            wt = sb.tile([B, 256], BF16, tag="wt")
            outsb = sb.tile([B, 10], F32, tag="outsb")

            idn = ba[:, IDN_C:IDN_C + 128]

            def patch(k):
                return ba[:, XB_C + k * 16:XB_C + (k + 1) * 16]

            # ---------- input DMAs ----------
            nc.sync.dma_start(ba[:], dBA[:, :])
            nc.sync.dma_start(e1g[0][:], dE1[:, 0:4096])
            nc.sync.dma_start(bb[:], dBB[:, :])
            for g in range(1, 4):
                nc.sync.dma_start(
                    e1g[g][:], dE1[:, g * 4096:(g + 1) * 4096]
                )

            # ---------- PE warm-up: release the HAM clock gate early -------
            wu = sb.tile([B, 128], BF16, tag="wu")
            nc.gpsimd.memset(wu[:], 0)
            wup = psH.tile([B, 512], F32, tag="hta", name="wup")
            for _ in range(30):
                nc.tensor.matmul(
                    wup[:, 0:128], wu[:], wu[:], start=True, stop=True
                )

            # ---------- Gcd = p9 (x) p10 ----------
            nc.vector.tensor_tensor(
                gcd[:].rearrange("p (c d) -> p c d", c=16),
                patch(9).unsqueeze(2).broadcast_to((B, 16, 16)),
                patch(10).unsqueeze(1).broadcast_to((B, 16, 16)),
                mul,
            )

            # P6D[s, b*128+j] = p6[s,b] * IDN[s,j], built in quarters
            def p6d_quarter(q):
                nc.vector.tensor_tensor(
                    p6d[:, q * 512:(q + 1) * 512].rearrange(
                        "p (b j) -> p b j", b=4
                    ),
                    idn.unsqueeze(1).broadcast_to((B, 4, 128)),
                    ba[:, XB_C + 96 + 4 * q:XB_C + 100 + 4 * q]
                    .unsqueeze(2)
                    .broadcast_to((B, 4, 128)),
                    mul,
                )

            # H^T via diag trick (PE); cast-copies split ACT / GpSimd
            def diag_quarter(q):
                for h in range(2):
                    pst = psH.tile([B, 512], F32, tag="hta")
                    nc.tensor.matmul(
                        pst[:],
                        gcd[:, h * 128:(h + 1) * 128],
                        p6d[:, q * 512:(q + 1) * 512],
                        start=True,
                        stop=True,
                    )
                    # scatter 4 b-blocks to chunk slots ch=2*(4q+r)+h
                    dst = bass.AP(
                        ht.tensor,
                        ht[:].offset + ((8 * q + h) * 128),
                        [ht[:].ap[0], [256, 4], [1, 128]],
                    )
                    src = pst[:].rearrange("p (r j) -> p r j", r=4)
                    nc.scalar.copy(dst, src)

            # ---------- iso stage: G_i builds ----------
            G_pairs = [(1, 4), (3, 7), (12, 13), (14, 15)]
            out_patches = [0, 2, 8, 11]
            for s, (pj, pk) in enumerate(G_pairs):
                nc.vector.tensor_tensor(
                    gi[:, s * 256:(s + 1) * 256].rearrange(
                        "p (j k) -> p j k", j=16
                    ),
                    patch(pj).unsqueeze(2).broadcast_to((B, 16, 16)),
                    patch(pk).unsqueeze(1).broadcast_to((B, 16, 16)),
                    mul,
                )

            p6d_quarter(0)
            diag_quarter(0)

            # ---------- G_i^T via PE transpose (batched 4 per PSUM tile) ---
            for t in range(2):
                pst = psT.tile([B, 512], BF16, tag="pst")
                for k in range(4):
                    nc.tensor.transpose(
                        pst[:, k * 128:(k + 1) * 128],
                        gi[:, (t * 4 + k) * 128:(t * 4 + k + 1) * 128],
                        idn,
                    )
                nc.scalar.copy(gt[:, t * 512:(t + 1) * 512], pst[:])

            p6d_quarter(1)
            diag_quarter(1)
            p6d_quarter(2)
            diag_quarter(2)
            p6d_quarter(3)
            diag_quarter(3)

            # ---------- Y matmuls ----------
            py = psY.tile([B, 1024], F32, tag="py")
            for s in range(4):
                for ch in range(2):
                    nc.tensor.matmul(
                        py[:, s * 256:(s + 1) * 256],
                        gt[:, (s * 2 + ch) * 128:(s * 2 + ch + 1) * 128],
                        bb[:, ch * 1024 + s * 256:ch * 1024 + (s + 1) * 256],
                        start=(ch == 0),
                        stop=(ch == 1),
                    )
            # ---------- z = Y * pout ; tall = reduce_i z  (layout (o,m)) ---
            for s, po in enumerate(out_patches):
                nc.vector.tensor_tensor(
                    zy[:, s * 256:(s + 1) * 256].rearrange(
                        "p (o m i) -> p o m i", o=4, m=4
                    ),
                    py[:, s * 256:(s + 1) * 256].rearrange(
                        "p (i m o) -> p o m i", i=16, m=4
                    ),
                    patch(po)
                    .unsqueeze(1)
                    .unsqueeze(1)
                    .broadcast_to((B, 4, 4, 16)),
                    mul,
                )
            nc.vector.tensor_reduce(
                tall[:],
                zy[:].rearrange("p (x i) -> p x i", i=16),
                axis=mybir.AxisListType.X,
                op=add,
            )
            # ---------- expansions: tx[s][p, (o, 64, m)] = t_s[m, o] -------
            nc.vector.tensor_copy(p5f[:], ba[:, P5_C:P5_C + 2])
            for s in range(4):
                src = bass.AP(
                    tall.tensor,
                    tall[:].offset + s * 16,
                    [tall[:].ap[0], [4, 4], [0, 64], [1, 4]],
                )
                dst = tx[s][:].rearrange("p (o x m) -> p o x m", o=4, x=64)
                nc.vector.tensor_copy(dst, src)

            # ---------- big matmul: pv[B, (a|fghe)] ------------------------
            pv = psV.tile([B, 512], F32, tag="pv")
            for g in range(4):
                for j in range(8):
                    nc.tensor.matmul(
                        pv[:],
                        ht[:, (8 * g + j) * 128:(8 * g + j + 1) * 128],
                        e1g[g][:, j * 512:(j + 1) * 512],
                        start=(g == 0 and j == 0),
                        stop=(g == 3 and j == 7),
                    )
            # v = p5[:,0]*v0 + p5[:,1]*v1
            nc.vector.tensor_scalar(
                vtmp[:], pv[:, 0:256], p5f[:, 0:1], None, mul
            )
            nc.vector.scalar_tensor_tensor(
                vsb[:], pv[:, 256:512], p5f[:, 1:2], vtmp[:],
                mul, add,
            )

            # ---------- tail: 4 mode products, all 2x-eligible -------------
            # each: Z[p, new, k, old] = A[p, k, old] * t[old, new]; reduce old
            def mode_step(dst, src, s):
                nc.vector.tensor_tensor(
                    zt[:].rearrange("p (n x o) -> p n x o", n=4, x=64),
                    src[:].rearrange("p (x o) -> p x o", o=4)
                    .unsqueeze(1)
                    .broadcast_to((B, 4, 64, 4)),
                    tx[s][:].rearrange("p (n x o) -> p n x o", n=4, x=64),
                    mul,
                )
                nc.vector.tensor_reduce(
                    dst[:],
                    zt[:].rearrange("p (x o) -> p x o", o=4),
                    axis=mybir.AxisListType.X,
                    op=add,
                )

            mode_step(a1, vsb, 0)   # contract e -> a ; A1 (a,f,g,h)
            mode_step(a2, a1, 3)    # contract h -> d ; A2 (d,a,f,g)
            mode_step(a3, a2, 2)    # contract g -> c ; A3 (c,d,a,f)
            mode_step(wsb, a3, 1)   # contract f -> b ; w  (b,c,d,a)

            # ---------- final: out = w^T.T @ M + bias ----------------------
            pwt = psT.tile([B, 512], BF16, tag="pst", name="pwt")
            for hf in range(2):
                nc.tensor.transpose(
                    pwt[:, hf * 128:(hf + 1) * 128],
                    wsb[:, hf * 128:(hf + 1) * 128],
                    idn,
                )
            nc.scalar.copy(wt[:], pwt[:, 0:256])
            po_ = psT.tile([B, 512], F32, tag="pst", name="po")
            for hf in range(2):
                nc.tensor.matmul(
                    po_[:, 0:10],
                    wt[:, hf * 128:(hf + 1) * 128],
                    ba[:, M_C + hf * 10:M_C + (hf + 1) * 10],
                    start=(hf == 0),
                    stop=(hf == 1),
                )
            nc.vector.tensor_tensor(
                outsb[:], po_[:, 0:10], ba[:, BIAS_C:BIAS_C + 10], add
            )
            nc.sync.dma_start(dOUT[:, :], outsb[:])

    nc.compile()
    return nc


def _host_prep(inputs, e1, e2, iso1_0, iso1_1, iso1_2, iso1_3, iso2, bias):
    """Layout prep: patch extraction, permutes, bf16 casts, per-core slices."""
    x = np.ascontiguousarray(np.asarray(inputs, np.float32))
    xv = (
        x[..., 0]
        .reshape(B, 4, 4, 4, 4)
        .transpose(0, 1, 3, 2, 4)
        .reshape(B, 16, 16)
    )
    Xb = xv.reshape(B, 256).astype(NPBF16)
    IDNb = np.eye(B, dtype=NPBF16)

    iso_perms = [
        (np.asarray(iso1_0, np.float32), (1, 2, 0, 3, 4)),
        (np.asarray(iso1_1, np.float32), (1, 3, 0, 2, 4)),
        (np.asarray(iso1_2, np.float32), (2, 3, 0, 1, 4)),
        (np.asarray(iso1_3, np.float32), (2, 3, 1, 0, 4)),
    ]
    ISO = np.concatenate(
        [t.transpose(p).reshape(256, 256) for t, p in iso_perms], axis=1
    ).astype(NPBF16)  # [256 (jk), 1024 (s, i, m, o)]

    M = (
        np.asarray(e2, np.float32).reshape(256, 256)
        @ np.asarray(iso2, np.float32).reshape(256, 10)
    )  # rows (a,b,c,d) -> permute to w layout (b,c,d,a)
    Mp = (
        M.reshape(4, 4, 4, 4, 10).transpose(1, 2, 3, 0, 4).reshape(256, 10)
    ).astype(NPBF16)

    # E1: [a,b,c,d,e,f,g,h] -> per-core [p=cd%128, cols g|j|a|f,g,h,e]
    E = np.asarray(e1, np.float32).reshape(8, 2, 4, 4, 2, 128, 4, 4, 4, 4)
    # axes: core, ar, bhi, blo, hh, p, e, f, g, h
    E = E.transpose(0, 5, 2, 3, 4, 1, 7, 8, 9, 6)
    E1all = np.ascontiguousarray(E).reshape(8, 128, 16384).astype(NPBF16)

    biasv = np.asarray(bias, np.float32).reshape(10)
    p5 = xv[:, 5, :]  # [B, 16]

    BB = np.ascontiguousarray(
        np.concatenate([ISO[0:128], ISO[128:256]], axis=1)
    )  # [128, 2048]

    in_maps = []
    for core in range(NCORES):
        a0 = 2 * core
        bac = np.zeros((B, BA_W), NPBF16)
        bac[:, XB_C:XB_C + 256] = Xb
        bac[:, IDN_C:IDN_C + 128] = IDNb
        bac[:, M_C:M_C + 10] = Mp[0:128]
        bac[:, M_C + 10:M_C + 20] = Mp[128:256]
        bac[:, P5_C:P5_C + 2] = p5[:, a0:a0 + 2].astype(NPBF16)
        if core == 0:
            bac[:, BIAS_C:BIAS_C + 10] = np.tile(biasv, (B, 1)).astype(
                NPBF16
            )
        in_maps.append(
            {
                "BA": np.ascontiguousarray(bac),
                "BB": BB,
                "E1": np.ascontiguousarray(E1all[core]),
            }
        )
    return in_maps


def kernel(inputs, e1, e2, iso1_0, iso1_1, iso1_2, iso1_3, iso2, bias):
    if "nc" not in _CACHE:
        _CACHE["nc"] = _build_nc()
    nc = _CACHE["nc"]
    in_maps = _host_prep(
        inputs, e1, e2, iso1_0, iso1_1, iso1_2, iso1_3, iso2, bias
    )
    res = bass_utils.run_bass_kernel_spmd(
        nc, in_maps, core_ids=list(range(NCORES))
    )
    out = np.zeros((B, 10), np.float32)
    for core in range(NCORES):
        out = out + res.results[core]["OUT"]
    return out.astype(np.float32)
